# revision 13
# baseline (speedup 1.0000x reference)
"""Trainium2 Bass kernel for FCOSPrototype segment-reduce + InfoNCE loss.

Computes, for inputs cls_feats [N,256], cls_targets [N], lvl_idx [N],
prototypes [17,5,256]:
  - fused segment-mean over seg = cls_targets*5 + lvl_idx  (85 segments)
  - InfoNCE loss between normalized prototypes and segment means

Strategy (8 NeuronCores, data-parallel over N):
  - each core streams its N/8 shard of cls_feats once, builds per-chunk
    one-hot matrices on DVE (seg == iota compare) and accumulates
    one-hot^T @ [x | 1 | 0] into PSUM on the PE (fp32r single-pass matmuls,
    exact for fp32r-rounded inputs) -> per-core sums[85,256] + counts[85]
  - AllReduce the [85,258] partials across the 8 cores
  - every core computes the tiny InfoNCE epilogue on-device; core 0's
    scalar loss is returned
"""

import numpy as np

import concourse.bacc as bacc
import concourse.bass as bass
import concourse.mybir as mybir
import concourse.tile as tile
from concourse import bass_utils
from concourse.masks import make_identity

# problem constants (hardcoded per contract)
N = 1_000_000
D = 256
C = 17
S = 5
NSEG = C * S  # 85
T = 0.07

NCORES = 8
P = 128
CHUNKS = 980          # chunks of 128 rows per core
G = 35                # chunks per DMA group
GROUPS = CHUNKS // G  # 28
ROWS_CORE = CHUNKS * P          # 125_440
N_PAD = NCORES * ROWS_CORE      # 1_003_520
DA = D + 2            # 258: [x | 1 | 0] -> even free dim (fp32r requirement)

F32 = mybir.dt.float32
F32R = mybir.dt.float32r

_CACHED_NC = None
_LAST_EXEC_NS = None


def _ensure_axon_ntff_hook():
    """Install the NTFF profile hook if the image lacks antenv.axon_hooks.

    Only affects tracing (BASS_TRACE=1); execution works without it.
    """
    try:
        from antenv.axon_hooks import get_axon_ntff_profile_hook  # noqa: F401
        return
    except ImportError:
        pass
    import sys as _sys
    import types as _types
    hook = None
    try:
        from trn_agent_boot.trn_boot import _ntff_profile_via_ctypes
        hook = _ntff_profile_via_ctypes("/opt/axon/libaxon_pjrt.so")
    except Exception:
        hook = None
    mod = _types.ModuleType("antenv.axon_hooks")
    mod._hook = hook
    mod.get_axon_ntff_profile_hook = lambda: mod._hook
    mod.set_axon_ntff_profile_hook = lambda h: setattr(mod, "_hook", h)
    _sys.modules["antenv.axon_hooks"] = mod
    try:
        import antenv
        antenv.axon_hooks = mod
    except ImportError:
        pass


_ensure_axon_ntff_hook()


def _round_fp32r(dst, src):
    """Round-to-nearest float32 -> float32r (low 12 mantissa bits zero)."""
    b = src.view(np.uint32).astype(np.uint64)
    r = (b + 0x7FF + ((b >> 12) & 1)) & 0xFFFFF000
    dst.view(np.uint32)[...] = r.astype(np.uint32)


def _build_nc():
    nc = bacc.Bacc("TRN2", target_bir_lowering=False, debug=False,
                   num_devices=NCORES)

    x_d = nc.dram_tensor("x", [ROWS_CORE, D], F32, kind="ExternalInput")
    seg_d = nc.dram_tensor("segt", [P, CHUNKS], F32, kind="ExternalInput")
    iota_d = nc.dram_tensor("iota", [P, G * NSEG], F32, kind="ExternalInput")
    proto_d = nc.dram_tensor("protos", [NSEG, D], F32, kind="ExternalInput")
    lab_d = nc.dram_tensor("labmask", [C, NSEG], F32, kind="ExternalInput")
    out_d = nc.dram_tensor("loss", [1, 1], F32, kind="ExternalOutput")

    with tile.TileContext(nc) as tc:
        with tc.tile_pool(name="sbuf", bufs=1) as sb, \
             tc.tile_pool(name="psum", bufs=1, space="PSUM") as ps, \
             tc.tile_pool(name="dram", bufs=1, space="DRAM") as dr:

            # ---- persistent tiles (small inputs go via SWDGE to keep
            # the two HWDGE rings free for the streaming loads) ---------
            seg_t = sb.tile([P, CHUNKS], F32, tag="seg_t")
            iota_t = sb.tile([P, G * NSEG], F32, tag="iota_t")
            nc.gpsimd.dma_start(seg_t[:], seg_d[:])
            nc.gpsimd.dma_start(iota_t[:], iota_d[:])

            NX = 2   # x-tile ring
            NO = 2   # one-hot ring
            x_tiles = [sb.tile([P, G * DA], F32R, name=f"xt{i}", tag=f"xt{i}")
                       for i in range(NX)]
            oh_tiles = [sb.tile([P, G * P], F32R, name=f"oh{i}", tag=f"oh{i}")
                        for i in range(NO)]
            x_r = x_d[:].rearrange("(g p) d -> p g d", p=P)  # [P, CHUNKS, D]
            iota3 = iota_t[:].rearrange("p (g j) -> p g j", g=G)

            # init constant regions of the f32r tiles via DVE (memset can't
            # write f32r; DVE output rounds to f32r which satisfies walrus)
            for t in x_tiles:
                # col 256 of each chunk = 1.0 (count column), col 257 = 0.0
                t3 = t[:].rearrange("p (g d) -> p g d", g=G)
                nc.vector.tensor_scalar(
                    out=t3[:, :, D:D + 1], in0=iota3[:, :, 0:1],
                    scalar1=0.0, scalar2=1.0,
                    op0=mybir.AluOpType.mult, op1=mybir.AluOpType.add)
                nc.vector.tensor_scalar(
                    out=t3[:, :, D + 1:DA], in0=iota3[:, :, 0:1],
                    scalar1=0.0, scalar2=None, op0=mybir.AluOpType.mult)
            for t in oh_tiles:
                # cols 85..127 of each chunk stay zero forever
                t3 = t[:].rearrange("p (g j) -> p g j", g=G)
                nc.vector.tensor_scalar(
                    out=t3[:, :, NSEG:P], in0=iota3[:, :, 0:P - NSEG],
                    scalar1=0.0, scalar2=None, op0=mybir.AluOpType.mult)

            acc = ps.tile([P, DA], F32, tag="acc", space="PSUM")

            # prototypes branch + ACT table warm-up: independent of the
            # streamed data, so schedule it up front where engines idle
            protos = sb.tile([NSEG, D], F32, tag="protos")
            nc.gpsimd.dma_start(protos[:], proto_d[:])
            lab = sb.tile([C, NSEG], F32, tag="lab")
            nc.gpsimd.dma_start(lab[:], lab_d[:])
            warm = sb.tile([1, 2], F32, tag="warm")
            for fn in (mybir.ActivationFunctionType.Square,
                       mybir.ActivationFunctionType.Sqrt,
                       mybir.ActivationFunctionType.Exp,
                       mybir.ActivationFunctionType.Ln,
                       mybir.ActivationFunctionType.Copy):
                nc.scalar.activation(out=warm[:], in_=iota_t[:1, :2], func=fn)

            def normalize(dst, src):
                sq = sb.tile([NSEG, D], F32, tag="nrm_sq")
                nc.scalar.activation(out=sq[:], in_=src,
                                     func=mybir.ActivationFunctionType.Square)
                ssum = sb.tile([NSEG, 1], F32, tag="nrm_ss")
                nc.vector.reduce_sum(out=ssum[:], in_=sq[:],
                                     axis=mybir.AxisListType.X)
                sq_root = sb.tile([NSEG, 1], F32, tag="nrm_sqrt")
                nc.scalar.activation(out=sq_root[:], in_=ssum[:],
                                     func=mybir.ActivationFunctionType.Sqrt)
                rs = sb.tile([NSEG, 1], F32, tag="nrm_rs")
                nc.vector.reciprocal(out=rs[:], in_=sq_root[:])
                nc.vector.tensor_scalar(out=dst[:], in0=src,
                                        scalar1=rs[:, :1], scalar2=None,
                                        op0=mybir.AluOpType.mult)

            ident = sb.tile([P, P], F32, tag="ident")
            make_identity(nc, ident[:])
            v1 = sb.tile([NSEG, D], F32, tag="v1")
            normalize(v1, protos[:])
            v1t = sb.tile([P, 2 * NSEG], F32, tag="v1t")
            v2t = sb.tile([P, 2 * NSEG], F32, tag="v2t")
            for h in range(2):
                pt = ps.tile([P, NSEG], F32, tag="ptrans", space="PSUM")
                nc.tensor.transpose(out=pt[:], in_=v1[:, h * P:(h + 1) * P],
                                    identity=ident[:NSEG, :NSEG])
                nc.vector.tensor_copy(out=v1t[:, h * NSEG:(h + 1) * NSEG],
                                      in_=pt[:])

            # ---- main streaming loop ----------------------------------
            for g in range(GROUPS):
                xt = x_tiles[g % NX]
                oh = oh_tiles[g % NO]
                xt3 = xt[:].rearrange("p (g d) -> p g d", g=G)
                oh3 = oh[:].rearrange("p (g j) -> p g j", g=G)

                # alternate the two HWDGE rings so transfers overlap
                dma_eng = nc.sync if g % 2 == 0 else nc.scalar
                dma_eng.dma_start(
                    xt3[:, :, :D],
                    x_r[:, g * G:(g + 1) * G, :].bitcast(F32R),
                )
                nc.vector.tensor_tensor(
                    out=oh3[:, :, :NSEG],
                    in0=seg_t[:, g * G:(g + 1) * G].to_broadcast([P, G, NSEG]),
                    in1=iota3[:],
                    op=mybir.AluOpType.is_equal,
                )
                for c in range(G):
                    k = g * G + c
                    nc.tensor.matmul(
                        out=acc[:],
                        lhsT=oh[:, c * P:(c + 1) * P],
                        rhs=xt[:, c * DA:(c + 1) * DA],
                        start=(k == 0),
                        stop=(k == CHUNKS - 1),
                    )

            # ---- partial [85, 258] -> AllReduce -----------------------
            part = sb.tile([NSEG, DA], F32, tag="part")
            nc.vector.tensor_copy(out=part[:], in_=acc[:NSEG, :])
            cc_in = dr.tile([NSEG, DA], F32)
            cc_out = dr.tile([NSEG, DA], F32)
            nc.sync.dma_start(cc_in[:], part[:])
            nc.gpsimd.collective_compute(
                "AllReduce", mybir.AluOpType.add,
                replica_groups=[list(range(NCORES))],
                ins=[cc_in.opt()], outs=[cc_out.opt()],
            )
            tot = sb.tile([NSEG, DA], F32, tag="tot")
            nc.sync.dma_start(tot[:], cc_out[:])

            # ---- epilogue: segment means + InfoNCE --------------------
            counts = tot[:, D:D + 1]                     # [85,1]
            cmax = sb.tile([NSEG, 1], F32, tag="cmax")
            nc.vector.tensor_scalar(out=cmax[:], in0=counts, scalar1=1.0,
                                    scalar2=None, op0=mybir.AluOpType.max)
            crec = sb.tile([NSEG, 1], F32, tag="crec")
            nc.vector.reciprocal(out=crec[:], in_=cmax[:])
            has = sb.tile([NSEG, 1], F32, tag="has")
            nc.vector.tensor_scalar(out=has[:], in0=counts, scalar1=0.0,
                                    scalar2=None, op0=mybir.AluOpType.is_gt)
            # delta = sums/max(counts,1); empty segments -> 0.01
            delta = sb.tile([NSEG, D], F32, tag="delta")
            nc.vector.tensor_scalar(out=delta[:], in0=tot[:, :D],
                                    scalar1=crec[:, :1], scalar2=None,
                                    op0=mybir.AluOpType.mult)
            blend = sb.tile([NSEG, 1], F32, tag="blend")
            nc.vector.tensor_scalar(out=blend[:], in0=has[:], scalar1=-0.01,
                                    scalar2=0.01, op0=mybir.AluOpType.mult,
                                    op1=mybir.AluOpType.add)
            deltaf = sb.tile([NSEG, D], F32, tag="deltaf")
            nc.vector.tensor_scalar(out=deltaf[:], in0=delta[:],
                                    scalar1=has[:, :1], scalar2=None,
                                    op0=mybir.AluOpType.mult)
            nc.vector.tensor_scalar(out=deltaf[:], in0=deltaf[:],
                                    scalar1=blend[:, :1], scalar2=None,
                                    op0=mybir.AluOpType.add)

            v2 = sb.tile([NSEG, D], F32, tag="v2")
            normalize(v2, deltaf[:])

            # transpose to [256(d on partitions), 85(cs)] in halves
            for h in range(2):
                pt = ps.tile([P, NSEG], F32, tag="ptrans", space="PSUM")
                nc.tensor.transpose(out=pt[:], in_=v2[:, h * P:(h + 1) * P],
                                    identity=ident[:NSEG, :NSEG])
                nc.vector.tensor_copy(out=v2t[:, h * NSEG:(h + 1) * NSEG],
                                      in_=pt[:])

            # logits[c, s*17+k] = sum_d v1[c,s,d] * v2[k,s,d]
            lg = ps.tile([C, NSEG], F32, tag="lg", space="PSUM")
            for s in range(S):
                for h in range(2):
                    nc.tensor.matmul(
                        out=lg[:, s * C:(s + 1) * C],
                        lhsT=v1t[:, h * NSEG + s:h * NSEG + NSEG:S],
                        rhs=v2t[:, h * NSEG + s:h * NSEG + NSEG:S],
                        start=(h == 0), stop=(h == 1),
                    )
            zl = sb.tile([C, NSEG], F32, tag="zl")
            nc.scalar.activation(out=zl[:], in_=lg[:],
                                 func=mybir.ActivationFunctionType.Copy,
                                 scale=1.0 / T)

            # masked cross-entropy over rows (c,s), labels k=(c*5+s)%17
            zl3 = zl[:].rearrange("c (s k) -> c s k", s=S)
            rmax = sb.tile([C, S], F32, tag="rmax")
            nc.vector.reduce_max(out=rmax[:], in_=zl3, axis=mybir.AxisListType.X)
            sh = sb.tile([C, NSEG], F32, tag="sh")
            sh3 = sh[:].rearrange("c (s k) -> c s k", s=S)
            nc.vector.tensor_tensor(out=sh3, in0=zl3,
                                    in1=rmax[:].to_broadcast([C, S, C]),
                                    op=mybir.AluOpType.subtract)
            ex = sb.tile([C, NSEG], F32, tag="ex")
            nc.scalar.activation(out=ex[:], in_=sh[:],
                                 func=mybir.ActivationFunctionType.Exp)
            se = sb.tile([C, S], F32, tag="se")
            nc.vector.reduce_sum(out=se[:],
                                 in_=ex[:].rearrange("c (s k) -> c s k", s=S),
                                 axis=mybir.AxisListType.X)
            lse = sb.tile([C, S], F32, tag="lse")
            nc.scalar.activation(out=lse[:], in_=se[:],
                                 func=mybir.ActivationFunctionType.Ln)
            pickt = sb.tile([C, NSEG], F32, tag="pickt")
            nc.vector.tensor_tensor(out=pickt[:], in0=sh[:], in1=lab[:],
                                    op=mybir.AluOpType.mult)
            pick = sb.tile([C, S], F32, tag="pick")
            nc.vector.reduce_sum(out=pick[:],
                                 in_=pickt[:].rearrange("c (s k) -> c s k", s=S),
                                 axis=mybir.AxisListType.X)
            pr = sb.tile([C, S], F32, tag="pr")
            nc.vector.tensor_tensor(out=pr[:], in0=lse[:], in1=pick[:],
                                    op=mybir.AluOpType.subtract)

            # mask [17,5] from allreduced counts (row cs = c*5+s)
            cnt17 = sb.tile([C, S], F32, tag="cnt17")
            nc.sync.dma_start(
                cnt17[:],
                cc_out[:].rearrange("(c s) d -> c s d", s=S)[:, :, D:D + 1],
            )
            has17 = sb.tile([C, S], F32, tag="has17")
            nc.vector.tensor_scalar(out=has17[:], in0=cnt17[:], scalar1=0.0,
                                    scalar2=None, op0=mybir.AluOpType.is_gt)
            masked = sb.tile([C, S], F32, tag="masked")
            nc.vector.tensor_tensor(out=masked[:], in0=pr[:], in1=has17[:],
                                    op=mybir.AluOpType.mult)
            pair = sb.tile([C, 2], F32, tag="pair")
            nc.vector.reduce_sum(out=pair[:, 0:1], in_=masked[:],
                                 axis=mybir.AxisListType.X)
            nc.vector.reduce_sum(out=pair[:, 1:2], in_=has17[:],
                                 axis=mybir.AxisListType.X)
            ones17 = sb.tile([C, 1], F32, tag="ones17")
            nc.vector.memset(ones17[:], 1.0)
            fin = ps.tile([1, 2], F32, tag="fin", space="PSUM")
            nc.tensor.matmul(out=fin[:], lhsT=ones17[:], rhs=pair[:],
                             start=True, stop=True)
            finsb = sb.tile([1, 2], F32, tag="finsb")
            nc.vector.tensor_copy(out=finsb[:], in_=fin[:])
            nmax = sb.tile([1, 1], F32, tag="nmax")
            nc.vector.tensor_scalar(out=nmax[:], in0=finsb[:, 1:2], scalar1=1.0,
                                    scalar2=None, op0=mybir.AluOpType.max)
            nrec = sb.tile([1, 1], F32, tag="nrec")
            nc.vector.reciprocal(out=nrec[:], in_=nmax[:])
            loss = sb.tile([1, 1], F32, tag="lossv")
            nc.vector.tensor_scalar(out=loss[:], in0=finsb[:, 0:1],
                                    scalar1=nrec[:, :1], scalar2=None,
                                    op0=mybir.AluOpType.mult)
            nc.sync.dma_start(out_d[:], loss[:])

    nc.compile()
    return nc


def _get_nc():
    global _CACHED_NC
    if _CACHED_NC is None:
        _CACHED_NC = _build_nc()
    return _CACHED_NC


def kernel(cls_feats, cls_targets, lvl_idx, prototypes):
    global _LAST_EXEC_NS
    cls_feats = np.ascontiguousarray(np.asarray(cls_feats, dtype=np.float32))
    cls_targets = np.asarray(cls_targets).astype(np.int64)
    lvl_idx = np.asarray(lvl_idx).astype(np.int64)
    prototypes = np.ascontiguousarray(np.asarray(prototypes, dtype=np.float32))

    n = cls_feats.shape[0]
    # features: pad to N_PAD rows and round to fp32r in blocks
    x = np.zeros((N_PAD, D), dtype=np.float32)
    blk = 1 << 16
    for i in range(0, n, blk):
        j = min(i + blk, n)
        _round_fp32r(x[i:j], cls_feats[i:j])

    # combined segment id; padding rows get -1 (never matches any segment)
    seg = np.full((N_PAD,), -1.0, dtype=np.float32)
    seg[:n] = (cls_targets * S + lvl_idx).astype(np.float32)

    iota = np.tile(np.arange(NSEG, dtype=np.float32), (P, G)).reshape(P, G * NSEG)
    # row c, col s*17+k = 1 iff k == (c*5+s) % 17
    cidx = np.arange(C)[:, None]
    sidx = np.arange(S)[None, :]
    kk = np.arange(C)[None, None, :]
    lab = ((cidx[:, :, None] * S + sidx[:, :, None]) % C == kk)
    lab = lab.astype(np.float32).reshape(C, NSEG)
    protos = prototypes.reshape(NSEG, D)

    in_maps = []
    for cix in range(NCORES):
        r0 = cix * ROWS_CORE
        seg_core = seg[r0:r0 + ROWS_CORE].reshape(CHUNKS, P).T
        in_maps.append({
            "x": x[r0:r0 + ROWS_CORE],
            "segt": np.ascontiguousarray(seg_core),
            "iota": iota,
            "protos": protos,
            "labmask": lab,
        })

    nc = _get_nc()
    res = bass_utils.run_bass_kernel_spmd(nc, in_maps,
                                          core_ids=list(range(NCORES)))
    _LAST_EXEC_NS = res.exec_time_ns
    global _LAST_RESULTS
    _LAST_RESULTS = res
    return np.float32(res.results[0]["loss"][0, 0])


_LAST_RESULTS = None


# revision 15
# speedup vs baseline: 1.0423x; 1.0423x over previous
"""Trainium2 Bass kernel for FCOSPrototype segment-reduce + InfoNCE loss.

Computes, for inputs cls_feats [N,256], cls_targets [N], lvl_idx [N],
prototypes [17,5,256]:
  - fused segment-mean over seg = cls_targets*5 + lvl_idx  (85 segments)
  - InfoNCE loss between normalized prototypes and segment means

Strategy (8 NeuronCores, data-parallel over N):
  - each core streams its N/8 shard of cls_feats once, builds per-chunk
    one-hot matrices on DVE (seg == iota compare) and accumulates
    one-hot^T @ [x | 1 | 0] into PSUM on the PE (fp32r single-pass matmuls,
    exact for fp32r-rounded inputs) -> per-core sums[85,256] + counts[85]
  - AllReduce the [85,258] partials across the 8 cores
  - every core computes the tiny InfoNCE epilogue on-device; core 0's
    scalar loss is returned
"""

import numpy as np

import concourse.bacc as bacc
import concourse.bass as bass
import concourse.mybir as mybir
import concourse.tile as tile
from concourse import bass_utils
from concourse.masks import make_identity

# problem constants (hardcoded per contract)
N = 1_000_000
D = 256
C = 17
S = 5
NSEG = C * S  # 85
T = 0.07

NCORES = 8
P = 128
CHUNKS = 980          # chunks of 128 rows per core
G = 28                # chunks per DMA group
GROUPS = CHUNKS // G  # 35
ROWS_CORE = CHUNKS * P          # 125_440
N_PAD = NCORES * ROWS_CORE      # 1_003_520
DA = D + 2            # 258: [x | 1 | 0] -> even free dim (fp32r requirement)

F32 = mybir.dt.float32
F32R = mybir.dt.float32r

_CACHED_NC = None
_LAST_EXEC_NS = None


def _ensure_axon_ntff_hook():
    """Install the NTFF profile hook if the image lacks antenv.axon_hooks.

    Only affects tracing (BASS_TRACE=1); execution works without it.
    """
    try:
        from antenv.axon_hooks import get_axon_ntff_profile_hook  # noqa: F401
        return
    except ImportError:
        pass
    import sys as _sys
    import types as _types
    hook = None
    try:
        from trn_agent_boot.trn_boot import _ntff_profile_via_ctypes
        hook = _ntff_profile_via_ctypes("/opt/axon/libaxon_pjrt.so")
    except Exception:
        hook = None
    mod = _types.ModuleType("antenv.axon_hooks")
    mod._hook = hook
    mod.get_axon_ntff_profile_hook = lambda: mod._hook
    mod.set_axon_ntff_profile_hook = lambda h: setattr(mod, "_hook", h)
    _sys.modules["antenv.axon_hooks"] = mod
    try:
        import antenv
        antenv.axon_hooks = mod
    except ImportError:
        pass


_ensure_axon_ntff_hook()


def _round_fp32r(dst, src):
    """Round-to-nearest float32 -> float32r (low 12 mantissa bits zero)."""
    b = src.view(np.uint32).astype(np.uint64)
    r = (b + 0x7FF + ((b >> 12) & 1)) & 0xFFFFF000
    dst.view(np.uint32)[...] = r.astype(np.uint32)


def _build_nc():
    nc = bacc.Bacc("TRN2", target_bir_lowering=False, debug=False,
                   num_devices=NCORES)

    x_d = nc.dram_tensor("x", [ROWS_CORE, D], F32, kind="ExternalInput")
    seg_d = nc.dram_tensor("segt", [P, CHUNKS], F32, kind="ExternalInput")
    iota_d = nc.dram_tensor("iota", [P, G * NSEG], F32, kind="ExternalInput")
    proto_d = nc.dram_tensor("protos", [NSEG, D], F32, kind="ExternalInput")
    lab_d = nc.dram_tensor("labmask", [C, NSEG], F32, kind="ExternalInput")
    out_d = nc.dram_tensor("loss", [1, 1], F32, kind="ExternalOutput")

    with tile.TileContext(nc) as tc:
        with tc.tile_pool(name="sbuf", bufs=1) as sb, \
             tc.tile_pool(name="psum", bufs=1, space="PSUM") as ps, \
             tc.tile_pool(name="dram", bufs=1, space="DRAM") as dr:

            # ---- persistent tiles (small inputs go via SWDGE to keep
            # the two HWDGE rings free for the streaming loads) ---------
            seg_t = sb.tile([P, CHUNKS], F32, tag="seg_t")
            iota_t = sb.tile([P, G * NSEG], F32, tag="iota_t")
            nc.gpsimd.dma_start(seg_t[:], seg_d[:])
            nc.gpsimd.dma_start(iota_t[:], iota_d[:])

            NX = 3   # x-tile ring
            NO = 2   # one-hot ring
            x_tiles = [sb.tile([P, G * DA], F32R, name=f"xt{i}", tag=f"xt{i}")
                       for i in range(NX)]
            oh_tiles = [sb.tile([P, G * P], F32R, name=f"oh{i}", tag=f"oh{i}")
                        for i in range(NO)]
            x_r = x_d[:].rearrange("(g p) d -> p g d", p=P)  # [P, CHUNKS, D]
            iota3 = iota_t[:].rearrange("p (g j) -> p g j", g=G)

            # init constant regions of the f32r tiles via DVE (memset can't
            # write f32r; DVE output rounds to f32r which satisfies walrus)
            for t in x_tiles:
                # col 256 of each chunk = 1.0 (count column), col 257 = 0.0
                t3 = t[:].rearrange("p (g d) -> p g d", g=G)
                nc.vector.tensor_scalar(
                    out=t3[:, :, D:D + 1], in0=iota3[:, :, 0:1],
                    scalar1=0.0, scalar2=1.0,
                    op0=mybir.AluOpType.mult, op1=mybir.AluOpType.add)
                nc.vector.tensor_scalar(
                    out=t3[:, :, D + 1:DA], in0=iota3[:, :, 0:1],
                    scalar1=0.0, scalar2=None, op0=mybir.AluOpType.mult)
            for t in oh_tiles:
                # cols 85..127 of each chunk stay zero forever
                t3 = t[:].rearrange("p (g j) -> p g j", g=G)
                nc.vector.tensor_scalar(
                    out=t3[:, :, NSEG:P], in0=iota3[:, :, 0:P - NSEG],
                    scalar1=0.0, scalar2=None, op0=mybir.AluOpType.mult)

            acc = ps.tile([P, DA], F32, tag="acc", space="PSUM")

            # prototypes branch + ACT table warm-up: independent of the
            # streamed data, so schedule it up front where engines idle
            protos = sb.tile([NSEG, D], F32, tag="protos")
            nc.gpsimd.dma_start(protos[:], proto_d[:])
            lab = sb.tile([C, NSEG], F32, tag="lab")
            nc.gpsimd.dma_start(lab[:], lab_d[:])
            warm = sb.tile([1, 2], F32, tag="warm")
            for fn in (mybir.ActivationFunctionType.Square,
                       mybir.ActivationFunctionType.Sqrt,
                       mybir.ActivationFunctionType.Exp,
                       mybir.ActivationFunctionType.Ln,
                       mybir.ActivationFunctionType.Copy):
                nc.scalar.activation(out=warm[:], in_=iota_t[:1, :2], func=fn)

            def normalize(dst, src):
                sq = sb.tile([NSEG, D], F32, tag="nrm_sq")
                nc.scalar.activation(out=sq[:], in_=src,
                                     func=mybir.ActivationFunctionType.Square)
                ssum = sb.tile([NSEG, 1], F32, tag="nrm_ss")
                nc.vector.reduce_sum(out=ssum[:], in_=sq[:],
                                     axis=mybir.AxisListType.X)
                sq_root = sb.tile([NSEG, 1], F32, tag="nrm_sqrt")
                nc.scalar.activation(out=sq_root[:], in_=ssum[:],
                                     func=mybir.ActivationFunctionType.Sqrt)
                rs = sb.tile([NSEG, 1], F32, tag="nrm_rs")
                nc.vector.reciprocal(out=rs[:], in_=sq_root[:])
                nc.vector.tensor_scalar(out=dst[:], in0=src,
                                        scalar1=rs[:, :1], scalar2=None,
                                        op0=mybir.AluOpType.mult)

            ident = sb.tile([P, P], F32, tag="ident")
            make_identity(nc, ident[:])
            v1 = sb.tile([NSEG, D], F32, tag="v1")
            normalize(v1, protos[:])
            v1t = sb.tile([P, 2 * NSEG], F32, tag="v1t")
            v2t = sb.tile([P, 2 * NSEG], F32, tag="v2t")
            for h in range(2):
                pt = ps.tile([P, NSEG], F32, tag="ptrans", space="PSUM")
                nc.tensor.transpose(out=pt[:], in_=v1[:, h * P:(h + 1) * P],
                                    identity=ident[:NSEG, :NSEG])
                nc.vector.tensor_copy(out=v1t[:, h * NSEG:(h + 1) * NSEG],
                                      in_=pt[:])

            # ---- main streaming loop ----------------------------------
            for g in range(GROUPS):
                xt = x_tiles[g % NX]
                oh = oh_tiles[g % NO]
                xt3 = xt[:].rearrange("p (g d) -> p g d", g=G)
                oh3 = oh[:].rearrange("p (g j) -> p g j", g=G)

                # alternate the two HWDGE rings so transfers overlap
                dma_eng = nc.sync if g % 2 == 0 else nc.scalar
                dma_eng.dma_start(
                    xt3[:, :, :D],
                    x_r[:, g * G:(g + 1) * G, :].bitcast(F32R),
                )
                nc.vector.tensor_tensor(
                    out=oh3[:, :, :NSEG],
                    in0=seg_t[:, g * G:(g + 1) * G].to_broadcast([P, G, NSEG]),
                    in1=iota3[:],
                    op=mybir.AluOpType.is_equal,
                )
                for c in range(G):
                    k = g * G + c
                    nc.tensor.matmul(
                        out=acc[:],
                        lhsT=oh[:, c * P:(c + 1) * P],
                        rhs=xt[:, c * DA:(c + 1) * DA],
                        start=(k == 0),
                        stop=(k == CHUNKS - 1),
                    )

            # ---- partial [85, 258] -> AllReduce -----------------------
            part = sb.tile([NSEG, DA], F32, tag="part")
            nc.vector.tensor_copy(out=part[:], in_=acc[:NSEG, :])
            cc_in = dr.tile([NSEG, DA], F32)
            cc_out = dr.tile([NSEG, DA], F32)
            nc.sync.dma_start(cc_in[:], part[:])
            nc.gpsimd.collective_compute(
                "AllReduce", mybir.AluOpType.add,
                replica_groups=[list(range(NCORES))],
                ins=[cc_in.opt()], outs=[cc_out.opt()],
            )
            tot = sb.tile([NSEG, DA], F32, tag="tot")
            nc.sync.dma_start(tot[:], cc_out[:])

            # ---- epilogue: segment means + InfoNCE --------------------
            counts = tot[:, D:D + 1]                     # [85,1]
            cmax = sb.tile([NSEG, 1], F32, tag="cmax")
            nc.vector.tensor_scalar(out=cmax[:], in0=counts, scalar1=1.0,
                                    scalar2=None, op0=mybir.AluOpType.max)
            crec = sb.tile([NSEG, 1], F32, tag="crec")
            nc.vector.reciprocal(out=crec[:], in_=cmax[:])
            has = sb.tile([NSEG, 1], F32, tag="has")
            nc.vector.tensor_scalar(out=has[:], in0=counts, scalar1=0.0,
                                    scalar2=None, op0=mybir.AluOpType.is_gt)
            # delta = sums/max(counts,1); empty segments -> 0.01
            delta = sb.tile([NSEG, D], F32, tag="delta")
            nc.vector.tensor_scalar(out=delta[:], in0=tot[:, :D],
                                    scalar1=crec[:, :1], scalar2=None,
                                    op0=mybir.AluOpType.mult)
            blend = sb.tile([NSEG, 1], F32, tag="blend")
            nc.vector.tensor_scalar(out=blend[:], in0=has[:], scalar1=-0.01,
                                    scalar2=0.01, op0=mybir.AluOpType.mult,
                                    op1=mybir.AluOpType.add)
            deltaf = sb.tile([NSEG, D], F32, tag="deltaf")
            nc.vector.tensor_scalar(out=deltaf[:], in0=delta[:],
                                    scalar1=has[:, :1], scalar2=None,
                                    op0=mybir.AluOpType.mult)
            nc.vector.tensor_scalar(out=deltaf[:], in0=deltaf[:],
                                    scalar1=blend[:, :1], scalar2=None,
                                    op0=mybir.AluOpType.add)

            v2 = sb.tile([NSEG, D], F32, tag="v2")
            normalize(v2, deltaf[:])

            # transpose to [256(d on partitions), 85(cs)] in halves
            for h in range(2):
                pt = ps.tile([P, NSEG], F32, tag="ptrans", space="PSUM")
                nc.tensor.transpose(out=pt[:], in_=v2[:, h * P:(h + 1) * P],
                                    identity=ident[:NSEG, :NSEG])
                nc.vector.tensor_copy(out=v2t[:, h * NSEG:(h + 1) * NSEG],
                                      in_=pt[:])

            # logits[c, s*17+k] = sum_d v1[c,s,d] * v2[k,s,d]
            lg = ps.tile([C, NSEG], F32, tag="lg", space="PSUM")
            for s in range(S):
                for h in range(2):
                    nc.tensor.matmul(
                        out=lg[:, s * C:(s + 1) * C],
                        lhsT=v1t[:, h * NSEG + s:h * NSEG + NSEG:S],
                        rhs=v2t[:, h * NSEG + s:h * NSEG + NSEG:S],
                        start=(h == 0), stop=(h == 1),
                    )
            zl = sb.tile([C, NSEG], F32, tag="zl")
            nc.scalar.activation(out=zl[:], in_=lg[:],
                                 func=mybir.ActivationFunctionType.Copy,
                                 scale=1.0 / T)

            # masked cross-entropy over rows (c,s), labels k=(c*5+s)%17
            zl3 = zl[:].rearrange("c (s k) -> c s k", s=S)
            rmax = sb.tile([C, S], F32, tag="rmax")
            nc.vector.reduce_max(out=rmax[:], in_=zl3, axis=mybir.AxisListType.X)
            sh = sb.tile([C, NSEG], F32, tag="sh")
            sh3 = sh[:].rearrange("c (s k) -> c s k", s=S)
            nc.vector.tensor_tensor(out=sh3, in0=zl3,
                                    in1=rmax[:].to_broadcast([C, S, C]),
                                    op=mybir.AluOpType.subtract)
            ex = sb.tile([C, NSEG], F32, tag="ex")
            nc.scalar.activation(out=ex[:], in_=sh[:],
                                 func=mybir.ActivationFunctionType.Exp)
            se = sb.tile([C, S], F32, tag="se")
            nc.vector.reduce_sum(out=se[:],
                                 in_=ex[:].rearrange("c (s k) -> c s k", s=S),
                                 axis=mybir.AxisListType.X)
            lse = sb.tile([C, S], F32, tag="lse")
            nc.scalar.activation(out=lse[:], in_=se[:],
                                 func=mybir.ActivationFunctionType.Ln)
            pickt = sb.tile([C, NSEG], F32, tag="pickt")
            nc.vector.tensor_tensor(out=pickt[:], in0=sh[:], in1=lab[:],
                                    op=mybir.AluOpType.mult)
            pick = sb.tile([C, S], F32, tag="pick")
            nc.vector.reduce_sum(out=pick[:],
                                 in_=pickt[:].rearrange("c (s k) -> c s k", s=S),
                                 axis=mybir.AxisListType.X)
            pr = sb.tile([C, S], F32, tag="pr")
            nc.vector.tensor_tensor(out=pr[:], in0=lse[:], in1=pick[:],
                                    op=mybir.AluOpType.subtract)

            # mask [17,5] from allreduced counts (row cs = c*5+s)
            cnt17 = sb.tile([C, S], F32, tag="cnt17")
            nc.sync.dma_start(
                cnt17[:],
                cc_out[:].rearrange("(c s) d -> c s d", s=S)[:, :, D:D + 1],
            )
            has17 = sb.tile([C, S], F32, tag="has17")
            nc.vector.tensor_scalar(out=has17[:], in0=cnt17[:], scalar1=0.0,
                                    scalar2=None, op0=mybir.AluOpType.is_gt)
            masked = sb.tile([C, S], F32, tag="masked")
            nc.vector.tensor_tensor(out=masked[:], in0=pr[:], in1=has17[:],
                                    op=mybir.AluOpType.mult)
            pair = sb.tile([C, 2], F32, tag="pair")
            nc.vector.reduce_sum(out=pair[:, 0:1], in_=masked[:],
                                 axis=mybir.AxisListType.X)
            nc.vector.reduce_sum(out=pair[:, 1:2], in_=has17[:],
                                 axis=mybir.AxisListType.X)
            ones17 = sb.tile([C, 1], F32, tag="ones17")
            nc.vector.memset(ones17[:], 1.0)
            fin = ps.tile([1, 2], F32, tag="fin", space="PSUM")
            nc.tensor.matmul(out=fin[:], lhsT=ones17[:], rhs=pair[:],
                             start=True, stop=True)
            finsb = sb.tile([1, 2], F32, tag="finsb")
            nc.vector.tensor_copy(out=finsb[:], in_=fin[:])
            nmax = sb.tile([1, 1], F32, tag="nmax")
            nc.vector.tensor_scalar(out=nmax[:], in0=finsb[:, 1:2], scalar1=1.0,
                                    scalar2=None, op0=mybir.AluOpType.max)
            nrec = sb.tile([1, 1], F32, tag="nrec")
            nc.vector.reciprocal(out=nrec[:], in_=nmax[:])
            loss = sb.tile([1, 1], F32, tag="lossv")
            nc.vector.tensor_scalar(out=loss[:], in0=finsb[:, 0:1],
                                    scalar1=nrec[:, :1], scalar2=None,
                                    op0=mybir.AluOpType.mult)
            nc.sync.dma_start(out_d[:], loss[:])

    nc.compile()
    return nc


def _get_nc():
    global _CACHED_NC
    if _CACHED_NC is None:
        _CACHED_NC = _build_nc()
    return _CACHED_NC


def kernel(cls_feats, cls_targets, lvl_idx, prototypes):
    global _LAST_EXEC_NS
    cls_feats = np.ascontiguousarray(np.asarray(cls_feats, dtype=np.float32))
    cls_targets = np.asarray(cls_targets).astype(np.int64)
    lvl_idx = np.asarray(lvl_idx).astype(np.int64)
    prototypes = np.ascontiguousarray(np.asarray(prototypes, dtype=np.float32))

    n = cls_feats.shape[0]
    # features: pad to N_PAD rows and round to fp32r in blocks
    x = np.zeros((N_PAD, D), dtype=np.float32)
    blk = 1 << 16
    for i in range(0, n, blk):
        j = min(i + blk, n)
        _round_fp32r(x[i:j], cls_feats[i:j])

    # combined segment id; padding rows get -1 (never matches any segment)
    seg = np.full((N_PAD,), -1.0, dtype=np.float32)
    seg[:n] = (cls_targets * S + lvl_idx).astype(np.float32)

    iota = np.tile(np.arange(NSEG, dtype=np.float32), (P, G)).reshape(P, G * NSEG)
    # row c, col s*17+k = 1 iff k == (c*5+s) % 17
    cidx = np.arange(C)[:, None]
    sidx = np.arange(S)[None, :]
    kk = np.arange(C)[None, None, :]
    lab = ((cidx[:, :, None] * S + sidx[:, :, None]) % C == kk)
    lab = lab.astype(np.float32).reshape(C, NSEG)
    protos = prototypes.reshape(NSEG, D)

    in_maps = []
    for cix in range(NCORES):
        r0 = cix * ROWS_CORE
        seg_core = seg[r0:r0 + ROWS_CORE].reshape(CHUNKS, P).T
        in_maps.append({
            "x": x[r0:r0 + ROWS_CORE],
            "segt": np.ascontiguousarray(seg_core),
            "iota": iota,
            "protos": protos,
            "labmask": lab,
        })

    nc = _get_nc()
    res = bass_utils.run_bass_kernel_spmd(nc, in_maps,
                                          core_ids=list(range(NCORES)))
    _LAST_EXEC_NS = res.exec_time_ns
    global _LAST_RESULTS
    _LAST_RESULTS = res
    return np.float32(res.results[0]["loss"][0, 0])


_LAST_RESULTS = None


# revision 16
# speedup vs baseline: 1.0709x; 1.0275x over previous
"""Trainium2 Bass kernel for FCOSPrototype segment-reduce + InfoNCE loss.

Computes, for inputs cls_feats [N,256], cls_targets [N], lvl_idx [N],
prototypes [17,5,256]:
  - fused segment-mean over seg = cls_targets*5 + lvl_idx  (85 segments)
  - InfoNCE loss between normalized prototypes and segment means

Strategy (8 NeuronCores, data-parallel over N):
  - each core streams its N/8 shard of cls_feats once, builds per-chunk
    one-hot matrices on DVE (seg == iota compare) and accumulates
    one-hot^T @ [x | 1 | 0] into PSUM on the PE (fp32r single-pass matmuls,
    exact for fp32r-rounded inputs) -> per-core sums[85,256] + counts[85]
  - AllReduce the [85,258] partials across the 8 cores
  - every core computes the tiny InfoNCE epilogue on-device; core 0's
    scalar loss is returned
"""

import numpy as np

import concourse.bacc as bacc
import concourse.bass as bass
import concourse.mybir as mybir
import concourse.tile as tile
from concourse import bass_utils
from concourse.masks import make_identity

# problem constants (hardcoded per contract)
N = 1_000_000
D = 256
C = 17
S = 5
NSEG = C * S  # 85
T = 0.07

NCORES = 8
P = 128
CHUNKS = 980          # chunks of 128 rows per core
G = 20                # chunks per DMA group
GROUPS = CHUNKS // G  # 49
ROWS_CORE = CHUNKS * P          # 125_440
N_PAD = NCORES * ROWS_CORE      # 1_003_520
DA = D + 2            # 258: [x | 1 | 0] -> even free dim (fp32r requirement)

F32 = mybir.dt.float32
F32R = mybir.dt.float32r

_CACHED_NC = None
_LAST_EXEC_NS = None


def _ensure_axon_ntff_hook():
    """Install the NTFF profile hook if the image lacks antenv.axon_hooks.

    Only affects tracing (BASS_TRACE=1); execution works without it.
    """
    try:
        from antenv.axon_hooks import get_axon_ntff_profile_hook  # noqa: F401
        return
    except ImportError:
        pass
    import sys as _sys
    import types as _types
    hook = None
    try:
        from trn_agent_boot.trn_boot import _ntff_profile_via_ctypes
        hook = _ntff_profile_via_ctypes("/opt/axon/libaxon_pjrt.so")
    except Exception:
        hook = None
    mod = _types.ModuleType("antenv.axon_hooks")
    mod._hook = hook
    mod.get_axon_ntff_profile_hook = lambda: mod._hook
    mod.set_axon_ntff_profile_hook = lambda h: setattr(mod, "_hook", h)
    _sys.modules["antenv.axon_hooks"] = mod
    try:
        import antenv
        antenv.axon_hooks = mod
    except ImportError:
        pass


_ensure_axon_ntff_hook()


def _round_fp32r(dst, src):
    """Round-to-nearest float32 -> float32r (low 12 mantissa bits zero)."""
    b = src.view(np.uint32).astype(np.uint64)
    r = (b + 0x7FF + ((b >> 12) & 1)) & 0xFFFFF000
    dst.view(np.uint32)[...] = r.astype(np.uint32)


def _build_nc():
    nc = bacc.Bacc("TRN2", target_bir_lowering=False, debug=False,
                   num_devices=NCORES)

    x_d = nc.dram_tensor("x", [ROWS_CORE, D], F32, kind="ExternalInput")
    seg_d = nc.dram_tensor("segt", [P, CHUNKS], F32, kind="ExternalInput")
    iota_d = nc.dram_tensor("iota", [P, G * NSEG], F32, kind="ExternalInput")
    proto_d = nc.dram_tensor("protos", [NSEG, D], F32, kind="ExternalInput")
    lab_d = nc.dram_tensor("labmask", [C, NSEG], F32, kind="ExternalInput")
    out_d = nc.dram_tensor("loss", [1, 1], F32, kind="ExternalOutput")

    with tile.TileContext(nc) as tc:
        with tc.tile_pool(name="sbuf", bufs=1) as sb, \
             tc.tile_pool(name="psum", bufs=1, space="PSUM") as ps, \
             tc.tile_pool(name="dram", bufs=1, space="DRAM") as dr:

            # ---- persistent tiles (small inputs go via SWDGE to keep
            # the two HWDGE rings free for the streaming loads) ---------
            seg_t = sb.tile([P, CHUNKS], F32, tag="seg_t")
            iota_t = sb.tile([P, G * NSEG], F32, tag="iota_t")
            nc.gpsimd.dma_start(seg_t[:], seg_d[:])
            nc.gpsimd.dma_start(iota_t[:], iota_d[:])

            NX = 5   # x-tile ring
            NO = 3   # one-hot ring
            x_tiles = [sb.tile([P, G * DA], F32R, name=f"xt{i}", tag=f"xt{i}")
                       for i in range(NX)]
            oh_tiles = [sb.tile([P, G * P], F32R, name=f"oh{i}", tag=f"oh{i}")
                        for i in range(NO)]
            x_r = x_d[:].rearrange("(g p) d -> p g d", p=P)  # [P, CHUNKS, D]
            iota3 = iota_t[:].rearrange("p (g j) -> p g j", g=G)

            # init constant regions of the f32r tiles via DVE (memset can't
            # write f32r; DVE output rounds to f32r which satisfies walrus)
            for t in x_tiles:
                # col 256 of each chunk = 1.0 (count column), col 257 = 0.0
                t3 = t[:].rearrange("p (g d) -> p g d", g=G)
                nc.vector.tensor_scalar(
                    out=t3[:, :, D:D + 1], in0=iota3[:, :, 0:1],
                    scalar1=0.0, scalar2=1.0,
                    op0=mybir.AluOpType.mult, op1=mybir.AluOpType.add)
                nc.vector.tensor_scalar(
                    out=t3[:, :, D + 1:DA], in0=iota3[:, :, 0:1],
                    scalar1=0.0, scalar2=None, op0=mybir.AluOpType.mult)
            for t in oh_tiles:
                # cols 85..127 of each chunk stay zero forever
                t3 = t[:].rearrange("p (g j) -> p g j", g=G)
                nc.vector.tensor_scalar(
                    out=t3[:, :, NSEG:P], in0=iota3[:, :, 0:P - NSEG],
                    scalar1=0.0, scalar2=None, op0=mybir.AluOpType.mult)

            acc = ps.tile([P, DA], F32, tag="acc", space="PSUM")

            # prototypes branch + ACT table warm-up: independent of the
            # streamed data, so schedule it up front where engines idle
            protos = sb.tile([NSEG, D], F32, tag="protos")
            nc.gpsimd.dma_start(protos[:], proto_d[:])
            lab = sb.tile([C, NSEG], F32, tag="lab")
            nc.gpsimd.dma_start(lab[:], lab_d[:])
            warm = sb.tile([1, 2], F32, tag="warm")
            for fn in (mybir.ActivationFunctionType.Square,
                       mybir.ActivationFunctionType.Sqrt,
                       mybir.ActivationFunctionType.Exp,
                       mybir.ActivationFunctionType.Ln,
                       mybir.ActivationFunctionType.Copy):
                nc.scalar.activation(out=warm[:], in_=iota_t[:1, :2], func=fn)

            def normalize(dst, src):
                sq = sb.tile([NSEG, D], F32, tag="nrm_sq")
                nc.scalar.activation(out=sq[:], in_=src,
                                     func=mybir.ActivationFunctionType.Square)
                ssum = sb.tile([NSEG, 1], F32, tag="nrm_ss")
                nc.vector.reduce_sum(out=ssum[:], in_=sq[:],
                                     axis=mybir.AxisListType.X)
                sq_root = sb.tile([NSEG, 1], F32, tag="nrm_sqrt")
                nc.scalar.activation(out=sq_root[:], in_=ssum[:],
                                     func=mybir.ActivationFunctionType.Sqrt)
                rs = sb.tile([NSEG, 1], F32, tag="nrm_rs")
                nc.vector.reciprocal(out=rs[:], in_=sq_root[:])
                nc.vector.tensor_scalar(out=dst[:], in0=src,
                                        scalar1=rs[:, :1], scalar2=None,
                                        op0=mybir.AluOpType.mult)

            ident = sb.tile([P, P], F32, tag="ident")
            make_identity(nc, ident[:])
            v1 = sb.tile([NSEG, D], F32, tag="v1")
            normalize(v1, protos[:])
            v1t = sb.tile([P, 2 * NSEG], F32, tag="v1t")
            v2t = sb.tile([P, 2 * NSEG], F32, tag="v2t")
            for h in range(2):
                pt = ps.tile([P, NSEG], F32, tag="ptrans", space="PSUM")
                nc.tensor.transpose(out=pt[:], in_=v1[:, h * P:(h + 1) * P],
                                    identity=ident[:NSEG, :NSEG])
                nc.vector.tensor_copy(out=v1t[:, h * NSEG:(h + 1) * NSEG],
                                      in_=pt[:])

            # ---- main streaming loop ----------------------------------
            for g in range(GROUPS):
                xt = x_tiles[g % NX]
                oh = oh_tiles[g % NO]
                xt3 = xt[:].rearrange("p (g d) -> p g d", g=G)
                oh3 = oh[:].rearrange("p (g j) -> p g j", g=G)

                nc.sync.dma_start(
                    xt3[:, :, :D],
                    x_r[:, g * G:(g + 1) * G, :].bitcast(F32R),
                )
                nc.vector.tensor_tensor(
                    out=oh3[:, :, :NSEG],
                    in0=seg_t[:, g * G:(g + 1) * G].to_broadcast([P, G, NSEG]),
                    in1=iota3[:],
                    op=mybir.AluOpType.is_equal,
                )
                for c in range(G):
                    k = g * G + c
                    nc.tensor.matmul(
                        out=acc[:],
                        lhsT=oh[:, c * P:(c + 1) * P],
                        rhs=xt[:, c * DA:(c + 1) * DA],
                        start=(k == 0),
                        stop=(k == CHUNKS - 1),
                    )

            # ---- partial [85, 258] -> AllReduce -----------------------
            part = sb.tile([NSEG, DA], F32, tag="part")
            nc.vector.tensor_copy(out=part[:], in_=acc[:NSEG, :])
            cc_in = dr.tile([NSEG, DA], F32)
            cc_out = dr.tile([NSEG, DA], F32)
            nc.sync.dma_start(cc_in[:], part[:])
            nc.gpsimd.collective_compute(
                "AllReduce", mybir.AluOpType.add,
                replica_groups=[list(range(NCORES))],
                ins=[cc_in.opt()], outs=[cc_out.opt()],
            )
            tot = sb.tile([NSEG, DA], F32, tag="tot")
            nc.sync.dma_start(tot[:], cc_out[:])

            # ---- epilogue: segment means + InfoNCE --------------------
            counts = tot[:, D:D + 1]                     # [85,1]
            cmax = sb.tile([NSEG, 1], F32, tag="cmax")
            nc.vector.tensor_scalar(out=cmax[:], in0=counts, scalar1=1.0,
                                    scalar2=None, op0=mybir.AluOpType.max)
            crec = sb.tile([NSEG, 1], F32, tag="crec")
            nc.vector.reciprocal(out=crec[:], in_=cmax[:])
            has = sb.tile([NSEG, 1], F32, tag="has")
            nc.vector.tensor_scalar(out=has[:], in0=counts, scalar1=0.0,
                                    scalar2=None, op0=mybir.AluOpType.is_gt)
            # delta = sums/max(counts,1); empty segments -> 0.01
            delta = sb.tile([NSEG, D], F32, tag="delta")
            nc.vector.tensor_scalar(out=delta[:], in0=tot[:, :D],
                                    scalar1=crec[:, :1], scalar2=None,
                                    op0=mybir.AluOpType.mult)
            blend = sb.tile([NSEG, 1], F32, tag="blend")
            nc.vector.tensor_scalar(out=blend[:], in0=has[:], scalar1=-0.01,
                                    scalar2=0.01, op0=mybir.AluOpType.mult,
                                    op1=mybir.AluOpType.add)
            deltaf = sb.tile([NSEG, D], F32, tag="deltaf")
            nc.vector.tensor_scalar(out=deltaf[:], in0=delta[:],
                                    scalar1=has[:, :1], scalar2=None,
                                    op0=mybir.AluOpType.mult)
            nc.vector.tensor_scalar(out=deltaf[:], in0=deltaf[:],
                                    scalar1=blend[:, :1], scalar2=None,
                                    op0=mybir.AluOpType.add)

            v2 = sb.tile([NSEG, D], F32, tag="v2")
            normalize(v2, deltaf[:])

            # transpose to [256(d on partitions), 85(cs)] in halves
            for h in range(2):
                pt = ps.tile([P, NSEG], F32, tag="ptrans", space="PSUM")
                nc.tensor.transpose(out=pt[:], in_=v2[:, h * P:(h + 1) * P],
                                    identity=ident[:NSEG, :NSEG])
                nc.vector.tensor_copy(out=v2t[:, h * NSEG:(h + 1) * NSEG],
                                      in_=pt[:])

            # logits[c, s*17+k] = sum_d v1[c,s,d] * v2[k,s,d]
            lg = ps.tile([C, NSEG], F32, tag="lg", space="PSUM")
            for s in range(S):
                for h in range(2):
                    nc.tensor.matmul(
                        out=lg[:, s * C:(s + 1) * C],
                        lhsT=v1t[:, h * NSEG + s:h * NSEG + NSEG:S],
                        rhs=v2t[:, h * NSEG + s:h * NSEG + NSEG:S],
                        start=(h == 0), stop=(h == 1),
                    )
            zl = sb.tile([C, NSEG], F32, tag="zl")
            nc.scalar.activation(out=zl[:], in_=lg[:],
                                 func=mybir.ActivationFunctionType.Copy,
                                 scale=1.0 / T)

            # masked cross-entropy over rows (c,s), labels k=(c*5+s)%17
            zl3 = zl[:].rearrange("c (s k) -> c s k", s=S)
            rmax = sb.tile([C, S], F32, tag="rmax")
            nc.vector.reduce_max(out=rmax[:], in_=zl3, axis=mybir.AxisListType.X)
            sh = sb.tile([C, NSEG], F32, tag="sh")
            sh3 = sh[:].rearrange("c (s k) -> c s k", s=S)
            nc.vector.tensor_tensor(out=sh3, in0=zl3,
                                    in1=rmax[:].to_broadcast([C, S, C]),
                                    op=mybir.AluOpType.subtract)
            ex = sb.tile([C, NSEG], F32, tag="ex")
            nc.scalar.activation(out=ex[:], in_=sh[:],
                                 func=mybir.ActivationFunctionType.Exp)
            se = sb.tile([C, S], F32, tag="se")
            nc.vector.reduce_sum(out=se[:],
                                 in_=ex[:].rearrange("c (s k) -> c s k", s=S),
                                 axis=mybir.AxisListType.X)
            lse = sb.tile([C, S], F32, tag="lse")
            nc.scalar.activation(out=lse[:], in_=se[:],
                                 func=mybir.ActivationFunctionType.Ln)
            pickt = sb.tile([C, NSEG], F32, tag="pickt")
            nc.vector.tensor_tensor(out=pickt[:], in0=sh[:], in1=lab[:],
                                    op=mybir.AluOpType.mult)
            pick = sb.tile([C, S], F32, tag="pick")
            nc.vector.reduce_sum(out=pick[:],
                                 in_=pickt[:].rearrange("c (s k) -> c s k", s=S),
                                 axis=mybir.AxisListType.X)
            pr = sb.tile([C, S], F32, tag="pr")
            nc.vector.tensor_tensor(out=pr[:], in0=lse[:], in1=pick[:],
                                    op=mybir.AluOpType.subtract)

            # mask [17,5] from allreduced counts (row cs = c*5+s)
            cnt17 = sb.tile([C, S], F32, tag="cnt17")
            nc.sync.dma_start(
                cnt17[:],
                cc_out[:].rearrange("(c s) d -> c s d", s=S)[:, :, D:D + 1],
            )
            has17 = sb.tile([C, S], F32, tag="has17")
            nc.vector.tensor_scalar(out=has17[:], in0=cnt17[:], scalar1=0.0,
                                    scalar2=None, op0=mybir.AluOpType.is_gt)
            masked = sb.tile([C, S], F32, tag="masked")
            nc.vector.tensor_tensor(out=masked[:], in0=pr[:], in1=has17[:],
                                    op=mybir.AluOpType.mult)
            pair = sb.tile([C, 2], F32, tag="pair")
            nc.vector.reduce_sum(out=pair[:, 0:1], in_=masked[:],
                                 axis=mybir.AxisListType.X)
            nc.vector.reduce_sum(out=pair[:, 1:2], in_=has17[:],
                                 axis=mybir.AxisListType.X)
            ones17 = sb.tile([C, 1], F32, tag="ones17")
            nc.vector.memset(ones17[:], 1.0)
            fin = ps.tile([1, 2], F32, tag="fin", space="PSUM")
            nc.tensor.matmul(out=fin[:], lhsT=ones17[:], rhs=pair[:],
                             start=True, stop=True)
            finsb = sb.tile([1, 2], F32, tag="finsb")
            nc.vector.tensor_copy(out=finsb[:], in_=fin[:])
            nmax = sb.tile([1, 1], F32, tag="nmax")
            nc.vector.tensor_scalar(out=nmax[:], in0=finsb[:, 1:2], scalar1=1.0,
                                    scalar2=None, op0=mybir.AluOpType.max)
            nrec = sb.tile([1, 1], F32, tag="nrec")
            nc.vector.reciprocal(out=nrec[:], in_=nmax[:])
            loss = sb.tile([1, 1], F32, tag="lossv")
            nc.vector.tensor_scalar(out=loss[:], in0=finsb[:, 0:1],
                                    scalar1=nrec[:, :1], scalar2=None,
                                    op0=mybir.AluOpType.mult)
            nc.sync.dma_start(out_d[:], loss[:])

    nc.compile()
    return nc


def _get_nc():
    global _CACHED_NC
    if _CACHED_NC is None:
        _CACHED_NC = _build_nc()
    return _CACHED_NC


def kernel(cls_feats, cls_targets, lvl_idx, prototypes):
    global _LAST_EXEC_NS
    cls_feats = np.ascontiguousarray(np.asarray(cls_feats, dtype=np.float32))
    cls_targets = np.asarray(cls_targets).astype(np.int64)
    lvl_idx = np.asarray(lvl_idx).astype(np.int64)
    prototypes = np.ascontiguousarray(np.asarray(prototypes, dtype=np.float32))

    n = cls_feats.shape[0]
    # features: pad to N_PAD rows and round to fp32r in blocks
    x = np.zeros((N_PAD, D), dtype=np.float32)
    blk = 1 << 16
    for i in range(0, n, blk):
        j = min(i + blk, n)
        _round_fp32r(x[i:j], cls_feats[i:j])

    # combined segment id; padding rows get -1 (never matches any segment)
    seg = np.full((N_PAD,), -1.0, dtype=np.float32)
    seg[:n] = (cls_targets * S + lvl_idx).astype(np.float32)

    iota = np.tile(np.arange(NSEG, dtype=np.float32), (P, G)).reshape(P, G * NSEG)
    # row c, col s*17+k = 1 iff k == (c*5+s) % 17
    cidx = np.arange(C)[:, None]
    sidx = np.arange(S)[None, :]
    kk = np.arange(C)[None, None, :]
    lab = ((cidx[:, :, None] * S + sidx[:, :, None]) % C == kk)
    lab = lab.astype(np.float32).reshape(C, NSEG)
    protos = prototypes.reshape(NSEG, D)

    in_maps = []
    for cix in range(NCORES):
        r0 = cix * ROWS_CORE
        seg_core = seg[r0:r0 + ROWS_CORE].reshape(CHUNKS, P).T
        in_maps.append({
            "x": x[r0:r0 + ROWS_CORE],
            "segt": np.ascontiguousarray(seg_core),
            "iota": iota,
            "protos": protos,
            "labmask": lab,
        })

    nc = _get_nc()
    res = bass_utils.run_bass_kernel_spmd(nc, in_maps,
                                          core_ids=list(range(NCORES)))
    _LAST_EXEC_NS = res.exec_time_ns
    global _LAST_RESULTS
    _LAST_RESULTS = res
    return np.float32(res.results[0]["loss"][0, 0])


_LAST_RESULTS = None


# revision 17
# speedup vs baseline: 1.1217x; 1.0474x over previous
"""Trainium2 Bass kernel for FCOSPrototype segment-reduce + InfoNCE loss.

Computes, for inputs cls_feats [N,256], cls_targets [N], lvl_idx [N],
prototypes [17,5,256]:
  - fused segment-mean over seg = cls_targets*5 + lvl_idx  (85 segments)
  - InfoNCE loss between normalized prototypes and segment means

Strategy (8 NeuronCores, data-parallel over N):
  - each core streams its N/8 shard of cls_feats once, builds per-chunk
    one-hot matrices on DVE (seg == iota compare) and accumulates
    one-hot^T @ [x | 1 | 0] into PSUM on the PE (fp32r single-pass matmuls,
    exact for fp32r-rounded inputs) -> per-core sums[85,256] + counts[85]
  - AllReduce the [85,258] partials across the 8 cores
  - every core computes the tiny InfoNCE epilogue on-device; core 0's
    scalar loss is returned
"""

import numpy as np

import concourse.bacc as bacc
import concourse.bass as bass
import concourse.mybir as mybir
import concourse.tile as tile
from concourse import bass_utils
from concourse.masks import make_identity

# problem constants (hardcoded per contract)
N = 1_000_000
D = 256
C = 17
S = 5
NSEG = C * S  # 85
T = 0.07

NCORES = 8
P = 128
CHUNKS = 980          # chunks of 128 rows per core
G = 20                # chunks per DMA group
GROUPS = CHUNKS // G  # 49
ROWS_CORE = CHUNKS * P          # 125_440
N_PAD = NCORES * ROWS_CORE      # 1_003_520
DA = D + 2            # 258: [x | 1 | 0] -> even free dim (fp32r requirement)

F32 = mybir.dt.float32
F32R = mybir.dt.float32r

_CACHED_NC = None
_LAST_EXEC_NS = None


def _ensure_axon_ntff_hook():
    """Install the NTFF profile hook if the image lacks antenv.axon_hooks.

    Only affects tracing (BASS_TRACE=1); execution works without it.
    """
    try:
        from antenv.axon_hooks import get_axon_ntff_profile_hook  # noqa: F401
        return
    except ImportError:
        pass
    import sys as _sys
    import types as _types
    hook = None
    try:
        from trn_agent_boot.trn_boot import _ntff_profile_via_ctypes
        hook = _ntff_profile_via_ctypes("/opt/axon/libaxon_pjrt.so")
    except Exception:
        hook = None
    mod = _types.ModuleType("antenv.axon_hooks")
    mod._hook = hook
    mod.get_axon_ntff_profile_hook = lambda: mod._hook
    mod.set_axon_ntff_profile_hook = lambda h: setattr(mod, "_hook", h)
    _sys.modules["antenv.axon_hooks"] = mod
    try:
        import antenv
        antenv.axon_hooks = mod
    except ImportError:
        pass


_ensure_axon_ntff_hook()


def _round_fp32r(dst, src):
    """Round-to-nearest float32 -> float32r (low 12 mantissa bits zero)."""
    b = src.view(np.uint32).astype(np.uint64)
    r = (b + 0x7FF + ((b >> 12) & 1)) & 0xFFFFF000
    dst.view(np.uint32)[...] = r.astype(np.uint32)


def _build_nc():
    nc = bacc.Bacc("TRN2", target_bir_lowering=False, debug=False,
                   num_devices=NCORES)

    x_d = nc.dram_tensor("x", [ROWS_CORE, D], F32, kind="ExternalInput")
    seg_d = nc.dram_tensor("segt", [P, CHUNKS], F32, kind="ExternalInput")
    iota_d = nc.dram_tensor("iota", [P, G * NSEG], F32, kind="ExternalInput")
    proto_d = nc.dram_tensor("protos", [NSEG, D], F32, kind="ExternalInput")
    lab_d = nc.dram_tensor("labmask", [C, NSEG], F32, kind="ExternalInput")
    out_d = nc.dram_tensor("loss", [1, 1], F32, kind="ExternalOutput")

    with tile.TileContext(nc) as tc:
        with tc.tile_pool(name="sbuf", bufs=1) as sb, \
             tc.tile_pool(name="psum", bufs=1, space="PSUM") as ps, \
             tc.tile_pool(name="dram", bufs=1, space="DRAM") as dr:

            # ---- persistent tiles (small inputs go via SWDGE to keep
            # the two HWDGE rings free for the streaming loads) ---------
            seg_t = sb.tile([P, CHUNKS], F32, tag="seg_t")
            iota_t = sb.tile([P, G * NSEG], F32, tag="iota_t")
            nc.gpsimd.dma_start(seg_t[:], seg_d[:])
            nc.gpsimd.dma_start(iota_t[:], iota_d[:])

            NX = 5   # x-tile ring
            NO = 3   # one-hot ring
            x_tiles = [sb.tile([P, G * DA], F32R, name=f"xt{i}", tag=f"xt{i}")
                       for i in range(NX)]
            oh_tiles = [sb.tile([P, G * P], F32R, name=f"oh{i}", tag=f"oh{i}")
                        for i in range(NO)]
            x_r = x_d[:].rearrange("(g p) d -> p g d", p=P)  # [P, CHUNKS, D]
            iota3 = iota_t[:].rearrange("p (g j) -> p g j", g=G)

            # init constant regions of the f32r tiles via DVE (memset can't
            # write f32r; DVE output rounds to f32r which satisfies walrus)
            for t in x_tiles:
                # col 256 of each chunk = 1.0 (count column), col 257 = 0.0
                t3 = t[:].rearrange("p (g d) -> p g d", g=G)
                nc.vector.tensor_scalar(
                    out=t3[:, :, D:D + 1], in0=iota3[:, :, 0:1],
                    scalar1=0.0, scalar2=1.0,
                    op0=mybir.AluOpType.mult, op1=mybir.AluOpType.add)
                nc.vector.tensor_scalar(
                    out=t3[:, :, D + 1:DA], in0=iota3[:, :, 0:1],
                    scalar1=0.0, scalar2=None, op0=mybir.AluOpType.mult)
            for t in oh_tiles:
                # cols 85..127 of each chunk stay zero forever
                t3 = t[:].rearrange("p (g j) -> p g j", g=G)
                nc.vector.tensor_scalar(
                    out=t3[:, :, NSEG:P], in0=iota3[:, :, 0:P - NSEG],
                    scalar1=0.0, scalar2=None, op0=mybir.AluOpType.mult)

            acc = ps.tile([P, DA], F32, tag="acc", space="PSUM")

            # prototypes branch + ACT table warm-up: independent of the
            # streamed data, so schedule it up front where engines idle
            protos = sb.tile([NSEG, D], F32, tag="protos")
            nc.gpsimd.dma_start(protos[:], proto_d[:])
            lab = sb.tile([C, NSEG], F32, tag="lab")
            nc.gpsimd.dma_start(lab[:], lab_d[:])
            warm = sb.tile([1, 2], F32, tag="warm")
            for fn in (mybir.ActivationFunctionType.Square,
                       mybir.ActivationFunctionType.Sqrt,
                       mybir.ActivationFunctionType.Exp,
                       mybir.ActivationFunctionType.Ln,
                       mybir.ActivationFunctionType.Copy):
                nc.scalar.activation(out=warm[:], in_=iota_t[:1, :2], func=fn)

            def normalize(dst, src):
                sq = sb.tile([NSEG, D], F32, tag="nrm_sq")
                nc.scalar.activation(out=sq[:], in_=src,
                                     func=mybir.ActivationFunctionType.Square)
                ssum = sb.tile([NSEG, 1], F32, tag="nrm_ss")
                nc.vector.reduce_sum(out=ssum[:], in_=sq[:],
                                     axis=mybir.AxisListType.X)
                sq_root = sb.tile([NSEG, 1], F32, tag="nrm_sqrt")
                nc.scalar.activation(out=sq_root[:], in_=ssum[:],
                                     func=mybir.ActivationFunctionType.Sqrt)
                rs = sb.tile([NSEG, 1], F32, tag="nrm_rs")
                nc.vector.reciprocal(out=rs[:], in_=sq_root[:])
                nc.vector.tensor_scalar(out=dst[:], in0=src,
                                        scalar1=rs[:, :1], scalar2=None,
                                        op0=mybir.AluOpType.mult)

            ident = sb.tile([P, P], F32, tag="ident")
            make_identity(nc, ident[:])
            v1 = sb.tile([NSEG, D], F32, tag="v1")
            normalize(v1, protos[:])
            v1t = sb.tile([P, 2 * NSEG], F32, tag="v1t")
            v2t = sb.tile([P, 2 * NSEG], F32, tag="v2t")
            for h in range(2):
                pt = ps.tile([P, NSEG], F32, tag="ptrans", space="PSUM")
                nc.tensor.transpose(out=pt[:], in_=v1[:, h * P:(h + 1) * P],
                                    identity=ident[:NSEG, :NSEG])
                nc.vector.tensor_copy(out=v1t[:, h * NSEG:(h + 1) * NSEG],
                                      in_=pt[:])

            # ---- main streaming loop ----------------------------------
            for g in range(GROUPS):
                xt = x_tiles[g % NX]
                oh = oh_tiles[g % NO]
                xt3 = xt[:].rearrange("p (g d) -> p g d", g=G)
                oh3 = oh[:].rearrange("p (g j) -> p g j", g=G)

                # alternate the two HWDGE rings: ring B's transfer covers
                # ring A's completion/issue latency (HBM BW is the cap)
                dma_eng = nc.sync if g % 2 == 0 else nc.scalar
                dma_eng.dma_start(
                    xt3[:, :, :D],
                    x_r[:, g * G:(g + 1) * G, :].bitcast(F32R),
                )
                nc.vector.tensor_tensor(
                    out=oh3[:, :, :NSEG],
                    in0=seg_t[:, g * G:(g + 1) * G].to_broadcast([P, G, NSEG]),
                    in1=iota3[:],
                    op=mybir.AluOpType.is_equal,
                )
                for c in range(G):
                    k = g * G + c
                    nc.tensor.matmul(
                        out=acc[:],
                        lhsT=oh[:, c * P:(c + 1) * P],
                        rhs=xt[:, c * DA:(c + 1) * DA],
                        start=(k == 0),
                        stop=(k == CHUNKS - 1),
                    )

            # ---- partial [85, 258] -> AllReduce -----------------------
            part = sb.tile([NSEG, DA], F32, tag="part")
            nc.vector.tensor_copy(out=part[:], in_=acc[:NSEG, :])
            cc_in = dr.tile([NSEG, DA], F32)
            cc_out = dr.tile([NSEG, DA], F32)
            nc.sync.dma_start(cc_in[:], part[:])
            nc.gpsimd.collective_compute(
                "AllReduce", mybir.AluOpType.add,
                replica_groups=[list(range(NCORES))],
                ins=[cc_in.opt()], outs=[cc_out.opt()],
            )
            tot = sb.tile([NSEG, DA], F32, tag="tot")
            nc.sync.dma_start(tot[:], cc_out[:])

            # ---- epilogue: segment means + InfoNCE --------------------
            counts = tot[:, D:D + 1]                     # [85,1]
            cmax = sb.tile([NSEG, 1], F32, tag="cmax")
            nc.vector.tensor_scalar(out=cmax[:], in0=counts, scalar1=1.0,
                                    scalar2=None, op0=mybir.AluOpType.max)
            crec = sb.tile([NSEG, 1], F32, tag="crec")
            nc.vector.reciprocal(out=crec[:], in_=cmax[:])
            has = sb.tile([NSEG, 1], F32, tag="has")
            nc.vector.tensor_scalar(out=has[:], in0=counts, scalar1=0.0,
                                    scalar2=None, op0=mybir.AluOpType.is_gt)
            # delta = sums/max(counts,1); empty segments -> 0.01
            delta = sb.tile([NSEG, D], F32, tag="delta")
            nc.vector.tensor_scalar(out=delta[:], in0=tot[:, :D],
                                    scalar1=crec[:, :1], scalar2=None,
                                    op0=mybir.AluOpType.mult)
            blend = sb.tile([NSEG, 1], F32, tag="blend")
            nc.vector.tensor_scalar(out=blend[:], in0=has[:], scalar1=-0.01,
                                    scalar2=0.01, op0=mybir.AluOpType.mult,
                                    op1=mybir.AluOpType.add)
            deltaf = sb.tile([NSEG, D], F32, tag="deltaf")
            nc.vector.tensor_scalar(out=deltaf[:], in0=delta[:],
                                    scalar1=has[:, :1], scalar2=None,
                                    op0=mybir.AluOpType.mult)
            nc.vector.tensor_scalar(out=deltaf[:], in0=deltaf[:],
                                    scalar1=blend[:, :1], scalar2=None,
                                    op0=mybir.AluOpType.add)

            v2 = sb.tile([NSEG, D], F32, tag="v2")
            normalize(v2, deltaf[:])

            # transpose to [256(d on partitions), 85(cs)] in halves
            for h in range(2):
                pt = ps.tile([P, NSEG], F32, tag="ptrans", space="PSUM")
                nc.tensor.transpose(out=pt[:], in_=v2[:, h * P:(h + 1) * P],
                                    identity=ident[:NSEG, :NSEG])
                nc.vector.tensor_copy(out=v2t[:, h * NSEG:(h + 1) * NSEG],
                                      in_=pt[:])

            # logits[c, s*17+k] = sum_d v1[c,s,d] * v2[k,s,d]
            lg = ps.tile([C, NSEG], F32, tag="lg", space="PSUM")
            for s in range(S):
                for h in range(2):
                    nc.tensor.matmul(
                        out=lg[:, s * C:(s + 1) * C],
                        lhsT=v1t[:, h * NSEG + s:h * NSEG + NSEG:S],
                        rhs=v2t[:, h * NSEG + s:h * NSEG + NSEG:S],
                        start=(h == 0), stop=(h == 1),
                    )
            zl = sb.tile([C, NSEG], F32, tag="zl")
            nc.scalar.activation(out=zl[:], in_=lg[:],
                                 func=mybir.ActivationFunctionType.Copy,
                                 scale=1.0 / T)

            # masked cross-entropy over rows (c,s), labels k=(c*5+s)%17
            zl3 = zl[:].rearrange("c (s k) -> c s k", s=S)
            rmax = sb.tile([C, S], F32, tag="rmax")
            nc.vector.reduce_max(out=rmax[:], in_=zl3, axis=mybir.AxisListType.X)
            sh = sb.tile([C, NSEG], F32, tag="sh")
            sh3 = sh[:].rearrange("c (s k) -> c s k", s=S)
            nc.vector.tensor_tensor(out=sh3, in0=zl3,
                                    in1=rmax[:].to_broadcast([C, S, C]),
                                    op=mybir.AluOpType.subtract)
            ex = sb.tile([C, NSEG], F32, tag="ex")
            nc.scalar.activation(out=ex[:], in_=sh[:],
                                 func=mybir.ActivationFunctionType.Exp)
            se = sb.tile([C, S], F32, tag="se")
            nc.vector.reduce_sum(out=se[:],
                                 in_=ex[:].rearrange("c (s k) -> c s k", s=S),
                                 axis=mybir.AxisListType.X)
            lse = sb.tile([C, S], F32, tag="lse")
            nc.scalar.activation(out=lse[:], in_=se[:],
                                 func=mybir.ActivationFunctionType.Ln)
            pickt = sb.tile([C, NSEG], F32, tag="pickt")
            nc.vector.tensor_tensor(out=pickt[:], in0=sh[:], in1=lab[:],
                                    op=mybir.AluOpType.mult)
            pick = sb.tile([C, S], F32, tag="pick")
            nc.vector.reduce_sum(out=pick[:],
                                 in_=pickt[:].rearrange("c (s k) -> c s k", s=S),
                                 axis=mybir.AxisListType.X)
            pr = sb.tile([C, S], F32, tag="pr")
            nc.vector.tensor_tensor(out=pr[:], in0=lse[:], in1=pick[:],
                                    op=mybir.AluOpType.subtract)

            # mask [17,5] from allreduced counts (row cs = c*5+s)
            cnt17 = sb.tile([C, S], F32, tag="cnt17")
            nc.sync.dma_start(
                cnt17[:],
                cc_out[:].rearrange("(c s) d -> c s d", s=S)[:, :, D:D + 1],
            )
            has17 = sb.tile([C, S], F32, tag="has17")
            nc.vector.tensor_scalar(out=has17[:], in0=cnt17[:], scalar1=0.0,
                                    scalar2=None, op0=mybir.AluOpType.is_gt)
            masked = sb.tile([C, S], F32, tag="masked")
            nc.vector.tensor_tensor(out=masked[:], in0=pr[:], in1=has17[:],
                                    op=mybir.AluOpType.mult)
            pair = sb.tile([C, 2], F32, tag="pair")
            nc.vector.reduce_sum(out=pair[:, 0:1], in_=masked[:],
                                 axis=mybir.AxisListType.X)
            nc.vector.reduce_sum(out=pair[:, 1:2], in_=has17[:],
                                 axis=mybir.AxisListType.X)
            ones17 = sb.tile([C, 1], F32, tag="ones17")
            nc.vector.memset(ones17[:], 1.0)
            fin = ps.tile([1, 2], F32, tag="fin", space="PSUM")
            nc.tensor.matmul(out=fin[:], lhsT=ones17[:], rhs=pair[:],
                             start=True, stop=True)
            finsb = sb.tile([1, 2], F32, tag="finsb")
            nc.vector.tensor_copy(out=finsb[:], in_=fin[:])
            nmax = sb.tile([1, 1], F32, tag="nmax")
            nc.vector.tensor_scalar(out=nmax[:], in0=finsb[:, 1:2], scalar1=1.0,
                                    scalar2=None, op0=mybir.AluOpType.max)
            nrec = sb.tile([1, 1], F32, tag="nrec")
            nc.vector.reciprocal(out=nrec[:], in_=nmax[:])
            loss = sb.tile([1, 1], F32, tag="lossv")
            nc.vector.tensor_scalar(out=loss[:], in0=finsb[:, 0:1],
                                    scalar1=nrec[:, :1], scalar2=None,
                                    op0=mybir.AluOpType.mult)
            nc.sync.dma_start(out_d[:], loss[:])

    nc.compile()
    return nc


def _get_nc():
    global _CACHED_NC
    if _CACHED_NC is None:
        _CACHED_NC = _build_nc()
    return _CACHED_NC


def kernel(cls_feats, cls_targets, lvl_idx, prototypes):
    global _LAST_EXEC_NS
    cls_feats = np.ascontiguousarray(np.asarray(cls_feats, dtype=np.float32))
    cls_targets = np.asarray(cls_targets).astype(np.int64)
    lvl_idx = np.asarray(lvl_idx).astype(np.int64)
    prototypes = np.ascontiguousarray(np.asarray(prototypes, dtype=np.float32))

    n = cls_feats.shape[0]
    # features: pad to N_PAD rows and round to fp32r in blocks
    x = np.zeros((N_PAD, D), dtype=np.float32)
    blk = 1 << 16
    for i in range(0, n, blk):
        j = min(i + blk, n)
        _round_fp32r(x[i:j], cls_feats[i:j])

    # combined segment id; padding rows get -1 (never matches any segment)
    seg = np.full((N_PAD,), -1.0, dtype=np.float32)
    seg[:n] = (cls_targets * S + lvl_idx).astype(np.float32)

    iota = np.tile(np.arange(NSEG, dtype=np.float32), (P, G)).reshape(P, G * NSEG)
    # row c, col s*17+k = 1 iff k == (c*5+s) % 17
    cidx = np.arange(C)[:, None]
    sidx = np.arange(S)[None, :]
    kk = np.arange(C)[None, None, :]
    lab = ((cidx[:, :, None] * S + sidx[:, :, None]) % C == kk)
    lab = lab.astype(np.float32).reshape(C, NSEG)
    protos = prototypes.reshape(NSEG, D)

    in_maps = []
    for cix in range(NCORES):
        r0 = cix * ROWS_CORE
        seg_core = seg[r0:r0 + ROWS_CORE].reshape(CHUNKS, P).T
        in_maps.append({
            "x": x[r0:r0 + ROWS_CORE],
            "segt": np.ascontiguousarray(seg_core),
            "iota": iota,
            "protos": protos,
            "labmask": lab,
        })

    nc = _get_nc()
    res = bass_utils.run_bass_kernel_spmd(nc, in_maps,
                                          core_ids=list(range(NCORES)))
    _LAST_EXEC_NS = res.exec_time_ns
    global _LAST_RESULTS
    _LAST_RESULTS = res
    return np.float32(res.results[0]["loss"][0, 0])


_LAST_RESULTS = None
